# revision 14
# baseline (speedup 1.0000x reference)
"""Trainium2 Bass kernel for a 2-layer LIF spiking network (DSQN forward).

Math (per batch b, feature h, timestep t; THR=1, snntorch reset='subtract'):
    cur1 = W1 @ x_t + b1                      (precomputable, no recurrence)
    mem1 <- beta1*mem1 + cur1 - H(mem1 - 1)   (reset uses PREVIOUS mem)
    spk1 = H(mem1 - 1)
    cur2 = W2 @ spk1 + b2
    mem2 <- beta2*mem2 + cur2 - H(mem2 - 1)
    spk2 = H(mem2 - 1)
    out_t = W3 @ spk2 + b3

Design (v2 - raw bass, no Tile):
  - Pure data parallel: batch 512 -> 64 per core across 8 cores.
  - Feature-major on chip: partitions = H (128), free = (t, b) columns.
  - ONE fused custom DVE op per timestep advances BOTH layers (layer 2
    lagged by LAG chunks): tile [128, 2x64]: cols 0:64 = mem1_t,
    64:128 = mem2_{t-8*LAG}.  beta selected per column-half via Idx>=64.
    1024 ops x ~320ns = the DVE wall (~330us), semaphore-free chain
    (raw bass; same-engine FIFO gives the ordering, unlike Tile which
    pays a ~300ns/op semaphore round-trip).
  - Currents live in PSUM, interleaved per step [cur1_t | cur2_t];
    biases are injected by K=1 matmuls so the DVE reads PSUM directly
    (measured free) and no ScalarE PSUM->SBUF copy is needed.
  - Chunk buffer is split [mem1 steps | mem2 steps] so the bulk SIGN
    extraction and the W2/W3 matmul rhs stay contiguous (strided rhs
    halves TensorE throughput; strided dst + strided DVE io are free).
  - Matmuls in bf16 hi/lo splits (exact +-1 trick for W2/W3):
    W1: w1h@xh + w1h@xl + w1l@xh, W2: split-2, W3 single bf16.
  - Output bias c3 is added by the ScalarE PSUM->SBUF out-copy (free).
"""

from contextlib import ExitStack

import numpy as np
import ml_dtypes

import concourse.bacc as bacc
import concourse.mybir as mybir
from concourse.bass_utils import run_bass_kernel_spmd
from concourse.dve_spec import Spec, Src0, Src1, C0, C1, C2, One, Idx, lower
from concourse.dve_uop import DveOpSpec
from concourse.dve_ops import DveOp, OPS, _CUSTOM_DVE_ROW_BASE, _SUB_OPCODE_FOR_NAME

F32 = mybir.dt.float32
BF16 = mybir.dt.bfloat16
AF = mybir.ActivationFunctionType

N_CORES = 8
H = 128
F = 128
A = 16
B_LOC = 64          # batch per core
TC = 8              # timesteps per chunk
LAG = 3             # layer-2 lag in chunks
COLS1 = TC * B_LOC  # 512: one layer's chunk columns
NPC = 3             # pscomb PSUM slots (2 banks each) + 2 x p3 = 8 banks
NXS = 4             # x chunk slots
NCB = 3             # chunk buffer slots
NSG = 5             # sgn slots (> LAG+1)
NOB = 4             # out buffer slots

# Precision config: number of bf16 split terms (0 = fp32 matmul)
W1_TERMS = 0
W2_TERMS = 0
NBIAS = 6           # combined bias matmul K (3 bf16 split levels x 2 layers)


def _register_fused_op() -> DveOp:
    """Fused 2-layer LIF step on a [128, 2*64] tile.

    out = Src0 * beta + Src1 - (Src0 > 1),
    beta[k] = s0[p] for k<64 (layer1), s1[p] for k>=64 (layer2).
    """
    name = "LIF_FUSED2_ANT"
    for o in OPS:
        if o.name == name:
            return o
    beta = C0 + (Idx >= C2) * (C1 - C0)
    body = Src0 * beta + Src1 - (Src0 > One)

    def ref(in0, in1, s0, s1, imm2):
        i0 = in0.reshape(in0.shape[0], -1)
        i1 = in1.reshape(in1.shape[0], -1)
        idx = np.arange(i0.shape[1])[None, :]
        b = np.where(idx >= imm2, s1, s0)
        return (i0 * b + i1 - (i0 > 1.0)).astype(np.float32).reshape(in0.shape)

    spec = Spec(body=body, reference=ref)
    shas = {
        ver: DveOpSpec(name=name, uops=lower(spec, ver=ver), rd1_en=True).sha(ver)
        for ver in ("v3", "v4")
    }
    op = DveOp(name, spec, subdim=False, uops_sha=shas)
    OPS.append(op)
    _SUB_OPCODE_FOR_NAME[name] = _CUSTOM_DVE_ROW_BASE + len(OPS) - 1
    return op


def build_program(T: int = 1024):
    assert T % TC == 0
    NCH = T // TC              # main chunks (mem1 timeline)
    VCH = NCH + LAG            # vector chunks incl. layer-2 drain
    OUT_SHIFT = 10             # sync issues out-DMA[c - OUT_SHIFT]
    fused = _register_fused_op()

    nc = bacc.Bacc("TRN2", target_bir_lowering=False, debug=False,
                   num_devices=N_CORES, disable_frame_to_traceback=True)

    # ---- DRAM ----
    if W1_TERMS == 0:
        x_d = nc.dram_tensor("x", (F, T, B_LOC), F32, kind="ExternalInput")
        w1_d = nc.dram_tensor("w1t", (F, H), F32, kind="ExternalInput")
        XD = 1
    else:
        xh_d = nc.dram_tensor("xh", (F, T, B_LOC), BF16, kind="ExternalInput")
        xl_d = nc.dram_tensor("xl", (F, T, B_LOC), BF16, kind="ExternalInput")
        w1h_d = nc.dram_tensor("w1h", (F, H), BF16, kind="ExternalInput")
        w1l_d = nc.dram_tensor("w1l", (F, H), BF16, kind="ExternalInput")
        XD = 2
    if W2_TERMS == 0:
        w2_d = nc.dram_tensor("w2t", (H, H), F32, kind="ExternalInput")
    else:
        w2s_d = [nc.dram_tensor(f"w2s{i}", (H, H), BF16, kind="ExternalInput")
                 for i in range(W2_TERMS)]
    w3_d = nc.dram_tensor("w3t", (H, A), BF16, kind="ExternalInput")
    m0_d = nc.dram_tensor("mem0", (2, H, B_LOC), F32, kind="ExternalInput")
    brows_d = nc.dram_tensor("brows", (NBIAS, H), BF16, kind="ExternalInput")
    bmask_d = nc.dram_tensor("bmask", (NBIAS, COLS1), BF16, kind="ExternalInput")
    beta1_d = nc.dram_tensor("beta1", (H, 1), F32, kind="ExternalInput")
    beta2_d = nc.dram_tensor("beta2", (H, 1), F32, kind="ExternalInput")
    c3_d = nc.dram_tensor("c3", (A, 1), F32, kind="ExternalInput")
    neg1_d = nc.dram_tensor("neg1", (H, 1), F32, kind="ExternalInput")
    out_d = nc.dram_tensor("out", (A, T, B_LOC), F32, kind="ExternalOutput")

    es = ExitStack()
    with es:
        block = es.enter_context(nc.Block())
        sems = [es.enter_context(nc.semaphore(n)) for n in
                ("s_pre", "s_x", "s_ps", "s_vec", "s_sc", "s_w3", "s_oc",
                 "s_od")]
        s_pre, s_x, s_ps, s_vec, s_sc, s_w3, s_oc, s_od = sems

        def sbuf(name, shape, dt):
            return es.enter_context(nc.sbuf_tensor(name, shape, dt))

        w3_s = sbuf("w3s", [H, A], BF16)
        brows_s = sbuf("brows_s", [NBIAS, H], BF16)
        bmask_s = sbuf("bmask_s", [NBIAS, COLS1], BF16)
        beta1_s = sbuf("beta1_s", [H, 1], F32)
        beta2_s = sbuf("beta2_s", [H, 1], F32)
        c3_s = sbuf("c3_s", [A, 1], F32)
        neg1_s = sbuf("neg1_s", [H, 1], F32)
        minit = sbuf("minit", [128, 2 * COLS1], F32)
        cb = sbuf("cb", [128, NCB * 2 * COLS1], F32)
        sgf = sbuf("sgf", [128, NSG * COLS1], F32)
        sgb = sbuf("sgb", [128, NSG * COLS1], BF16)
        ob = sbuf("ob", [A, NOB * COLS1], F32)
        pc0 = es.enter_context(nc.psum_tensor("pc0", [128, 2 * COLS1], F32))
        pc1 = es.enter_context(nc.psum_tensor("pc1", [128, 2 * COLS1], F32))
        pc2 = es.enter_context(nc.psum_tensor("pc2", [128, 2 * COLS1], F32))
        p30 = es.enter_context(nc.psum_tensor("p30", [128, COLS1], F32))
        p31 = es.enter_context(nc.psum_tensor("p31", [128, COLS1], F32))
        pre_dmas = []

        def pre(dst, src):
            pre_dmas.append((dst, src))

        if W1_TERMS == 0:
            w1_s = sbuf("w1s", [F, H], F32)
            x_s = sbuf("xs", [F, NXS * COLS1], F32)
            pre(w1_s[:], w1_d.ap())
        else:
            w1h_s = sbuf("w1hs", [F, H], BF16)
            w1l_s = sbuf("w1ls", [F, H], BF16)
            xh_s = sbuf("xhs", [F, NXS * COLS1], BF16)
            xl_s = sbuf("xls", [F, NXS * COLS1], BF16)
            pre(w1h_s[:], w1h_d.ap())
            pre(w1l_s[:], w1l_d.ap())
        if W2_TERMS == 0:
            w2_s = sbuf("w2fs", [H, H], F32)
            pre(w2_s[:], w2_d.ap())
            w2list = [w2_s]
        else:
            w2list = []
            for i in range(W2_TERMS):
                t = sbuf(f"w2s{i}s", [H, H], BF16)
                w2list.append(t)
                pre(t[:], w2s_d[i].ap())
        pre(w3_s[:], w3_d.ap())
        pre(brows_s[:], brows_d.ap())
        pre(bmask_s[:], bmask_d.ap())
        pre(beta1_s[:], beta1_d.ap())
        pre(beta2_s[:], beta2_d.ap())
        pre(c3_s[:], c3_d.ap())
        pre(neg1_s[:], neg1_d.ap())
        # initial membranes into the strided carry layout: step-7 blocks
        pre(minit[:, 7 * B_LOC:8 * B_LOC], m0_d.ap()[0])
        pre(minit[:, COLS1 + 7 * B_LOC:COLS1 + 8 * B_LOC], m0_d.ap()[1])
        PRE = 16 * len(pre_dmas)

        pcs = [pc0, pc1, pc2]
        p3s = [p30, p31]

        # cb as [p, slot, half, step, col]
        cb5 = cb.ap().rearrange("p (s h u c) -> p s h u c",
                                s=NCB, h=2, u=TC, c=B_LOC)
        mi4 = minit.ap().rearrange("p (h u c) -> p h u c", h=2, u=TC, c=B_LOC)

        def carry_ap(slot, t):
            return cb5[:, slot, :, t, :]

        def pc_dst(pcten, half, hb):
            # [p, step, 64] strided dst for matmuls (even/odd 64-blocks),
            # restricted to PSUM bank hb (a matmul dst cannot span banks)
            return pcten.ap().rearrange("p (hb t h c) -> p hb t h c",
                                        hb=2, t=TC // 2, h=2,
                                        c=B_LOC)[:, hb, :, half, :]

        def x_chunk(xd, c):
            return xd.ap()[:, c * TC:(c + 1) * TC, :].rearrange(
                "p a b -> p (a b)")

        def sg1_ap(c):
            o = (c % NSG) * COLS1
            return sgf.ap()[:, o:o + COLS1]

        def sg2_ap(c):
            o = (c % NSG) * COLS1
            return sgb.ap()[:, o:o + COLS1]

        def out_dma(sync, o):
            sync.wait_ge(s_oc, o + 1)
            sync.dma_start(
                out_d.ap()[:, o * TC:(o + 1) * TC, :].rearrange(
                    "p a b -> p (a b)"),
                ob[:, (o % NOB) * COLS1:(o % NOB + 1) * COLS1],
            ).then_inc(s_od, 16)

        # ---------------- SYNC (DMA) ----------------
        @block.sync
        def _(sync):
            for dst, src in pre_dmas:
                sync.dma_start(dst, src).then_inc(s_pre, 16)
            for c in range(NCH):
                if c >= NXS:
                    sync.wait_ge(s_ps, c - NXS + 1)
                xs = slice((c % NXS) * COLS1, (c % NXS + 1) * COLS1)
                if W1_TERMS == 0:
                    sync.dma_start(x_s[:, xs], x_chunk(x_d, c)).then_inc(s_x, 16)
                else:
                    sync.dma_start(xh_s[:, xs], x_chunk(xh_d, c)).then_inc(s_x, 16)
                    sync.dma_start(xl_s[:, xs], x_chunk(xl_d, c)).then_inc(s_x, 16)
                if c - OUT_SHIFT >= 0:
                    out_dma(sync, c - OUT_SHIFT)
            for o in range(max(0, NCH - OUT_SHIFT), NCH):
                out_dma(sync, o)

        # ---------------- TENSOR ----------------
        @block.tensor
        def _(tensor):
            tensor.wait_ge(s_pre, PRE)
            n_w3 = 0

            def w3_mm(k):
                tensor.wait_ge(s_sc, k + 1)          # sgn2[k] ready
                if k - 4 > 0:
                    tensor.wait_ge(s_oc, k - 4)      # p3 slot free
                nc.tensor.matmul(p3s[k % 2].ap()[0:A, :], w3_s[:], sg2_ap(k),
                                 start=True, stop=True).then_inc(s_w3, 1)

            for c in range(VCH):
                pc = pcs[c % NPC]
                HB = COLS1 // 2  # 256 columns per PSUM bank per half
                if c < NCH:
                    tensor.wait_ge(s_x, 16 * XD * (c + 1))
                if c >= NPC:
                    tensor.wait_ge(s_vec, c - NPC + 1)  # pscomb slot free
                if c >= LAG:
                    tensor.wait_ge(s_sc, c - LAG + 1)   # sgn1[c-LAG] ready
                ins = None
                for hb in range(2):
                    bank = pc.ap()[:, hb * COLS1:(hb + 1) * COLS1]
                    # combined bias fill for both layers (starts the group)
                    ins = nc.tensor.matmul(bank, brows_s[:, :], bmask_s[:, :],
                                           start=True,
                                           stop=(c >= NCH and c < LAG))
                    x0 = (c % NXS) * COLS1 + hb * HB
                    hs_ = slice(hb * HB, (hb + 1) * HB)
                    if c < NCH:
                        ins = nc.tensor.matmul(pc_dst(pc, 0, hb), w1_s[:],
                                               x_s[:, x0:x0 + HB],
                                               start=False, stop=(c < LAG))
                    if c >= LAG:
                        sg1c = sg1_ap(c - LAG)
                        ins = nc.tensor.matmul(pc_dst(pc, 1, hb), w2_s[:],
                                               sg1c[:, hs_],
                                               start=False, stop=True)
                ins.then_inc(s_ps, 1)
                # ---- W3 for k = c-3 (lagged so its sgn2 wait is already
                # satisfied by the bank-1 wait and never blocks the pscomb
                # stream the vector chain depends on) ----
                k = c - 3
                if k >= LAG:
                    w3_mm(k)
                    n_w3 += 1
            for k in (VCH - 3, VCH - 2, VCH - 1):
                w3_mm(k)
                n_w3 += 1
            assert n_w3 == NCH

        # ---------------- VECTOR (the serial LIF chain) ----------------
        @block.vector
        def _(vector):
            vector.wait_ge(s_pre, PRE)
            for c in range(VCH):
                vector.wait_ge(s_ps, c + 1)
                if c >= NCB:
                    vector.wait_ge(s_sc, c - NCB + 2)  # cb slot free
                pc = pcs[c % NPC]
                slot = c % NCB
                for t in range(TC):
                    if t == 0:
                        src = (mi4[:, :, TC - 1, :] if c == 0
                               else carry_ap((c - 1) % NCB, TC - 1))
                    else:
                        src = carry_ap(slot, t - 1)
                    ins = nc.vector._custom_dve(
                        fused,
                        out=carry_ap(slot, t),
                        in0=src,
                        in1=pc.ap()[:, t * 2 * B_LOC:(t + 1) * 2 * B_LOC],
                        s0=beta1_s[:, 0:1],
                        s1=beta2_s[:, 0:1],
                        imm2=float(B_LOC),
                    )
                if c == LAG - 1:
                    # layer-2 carry re-init: garbage evolved during the lag
                    # warmup; restore the true initial mem2 before chunk LAG
                    ins = nc.vector.tensor_copy(
                        cb5[:, slot, 1, TC - 1, :], mi4[:, 1, TC - 1, :])
                ins.then_inc(s_vec, 1)

        # ---------------- SCALAR ----------------
        @block.scalar
        def _(scalar):
            scalar.wait_ge(s_pre, PRE)
            n_oc = 0

            def outcopy(k):
                o = k - LAG
                scalar.wait_ge(s_w3, k - LAG + 1)     # p3[k] written
                if o - NOB >= 0:
                    scalar.wait_ge(s_od, 16 * (o - NOB + 1))  # ob slot free
                nc.scalar.activation(
                    ob[:, (o % NOB) * COLS1:(o % NOB + 1) * COLS1],
                    p3s[k % 2].ap()[0:A, :],
                    AF.Identity, bias=c3_s[:, 0:1]).then_inc(s_oc, 1)

            for c in range(VCH):
                scalar.wait_ge(s_vec, c + 1)
                if c >= NSG:
                    scalar.wait_ge(s_ps, c - NSG + LAG + 2)  # sg slot free
                slot = c % NCB
                ins = None
                if c < NCH:
                    ins = nc.scalar.activation(
                        sg1_ap(c),
                        cb5[:, slot, 0, :, :].rearrange("p u c -> p (u c)"),
                        AF.Sign, bias=neg1_s[:, 0:1])
                if c >= LAG:
                    ins = nc.scalar.activation(
                        sg2_ap(c),
                        cb5[:, slot, 1, :, :].rearrange("p u c -> p (u c)"),
                        AF.Sign, bias=neg1_s[:, 0:1])
                assert ins is not None
                ins.then_inc(s_sc, 1)
                # out-copy for k = c-3 (mirrors the tensor W3 lag)
                k = c - 3
                if k >= LAG:
                    outcopy(k)
                    n_oc += 1
            for k in (VCH - 3, VCH - 2, VCH - 1):
                outcopy(k)
                n_oc += 1
            assert n_oc == NCH

    nc.compile()
    # Path-independent debug info so the neuron compile cache hits across
    # working directories.
    fixed_dbg = mybir.OpDebugInfo(filename="kernel.py", lineno=0,
                                  kernel_name="build_program:")
    _dbg_cache = {}

    def _sanitize(dbg):
        if dbg is None:
            return None
        key = (dbg.op_name, dbg.lineno, dbg.bass_funcname, dbg.kernel_name,
               dbg.ant_layer, dbg.ant_annotation)
        if key not in _dbg_cache:
            _dbg_cache[key] = mybir.OpDebugInfo(
                op_name=dbg.op_name, filename="kernel.py", lineno=dbg.lineno,
                bass_funcname=dbg.bass_funcname, kernel_name=dbg.kernel_name,
                ant_layer=dbg.ant_layer, ant_annotation=dbg.ant_annotation)
        return _dbg_cache[key]

    for fn in nc.m.functions:
        for alloc in fn.allocations:
            for ml in (getattr(alloc, "memorylocations", None) or []):
                if getattr(ml, "ant_debug", None) is not None:
                    ml.ant_debug = fixed_dbg
        for blk in fn.blocks:
            for inst in blk.instructions:
                inst.debug = _sanitize(inst.debug)
    return nc


def _bf16_split(a, terms):
    """Split fp32 array into `terms` bf16 arrays summing to ~a."""
    outs = []
    rem = np.asarray(a, np.float32)
    for _ in range(terms):
        h = rem.astype(ml_dtypes.bfloat16)
        outs.append(h)
        rem = (rem - h.astype(np.float32)).astype(np.float32)
    return outs


def make_in_maps(state_batch, hidden_states, W1, b1, beta1, W2, b2, beta2,
                 W3, b3, T=None):
    x = np.asarray(state_batch, np.float32)
    hs = np.asarray(hidden_states, np.float32)
    if T is None:
        T = x.shape[1]
    W1 = np.asarray(W1, np.float32)
    W2 = np.asarray(W2, np.float32)
    W3 = np.asarray(W3, np.float32)

    w1t = np.ascontiguousarray(W1.T)                       # (F,H)
    w2t = np.ascontiguousarray((0.5 * W2).T)               # (H,H)
    w3t = np.ascontiguousarray((0.5 * W3).T).astype(ml_dtypes.bfloat16)
    be1 = np.clip(np.asarray(beta1, np.float32), 0.0, 1.0).reshape(H, 1)
    be2 = np.clip(np.asarray(beta2, np.float32), 0.0, 1.0).reshape(H, 1)
    b1row = np.asarray(b1, np.float32).reshape(H)
    b2row = (np.asarray(b2, np.float64)
             + 0.5 * np.asarray(W2, np.float64).sum(1)).astype(
                 np.float32).reshape(H)
    brows = np.stack(_bf16_split(b1row, 3) + _bf16_split(b2row, 3))  # (6,H)
    blk = (np.arange(COLS1) // B_LOC) % 2
    bmask = np.zeros((NBIAS, COLS1), np.float32)
    bmask[0:3] = (blk == 0).astype(np.float32)[None, :]
    bmask[3:6] = (blk == 1).astype(np.float32)[None, :]
    bmask = bmask.astype(ml_dtypes.bfloat16)
    c3 = (np.asarray(b3, np.float64)
          + 0.5 * np.asarray(W3, np.float64).sum(1)).astype(
              np.float32).reshape(A, 1)
    neg1 = np.full((H, 1), -1.0, np.float32)

    common = {
        "w3t": w3t, "brows": brows, "bmask": bmask,
        "beta1": be1, "beta2": be2, "c3": c3, "neg1": neg1,
    }
    if W1_TERMS == 0:
        common["w1t"] = w1t
    else:
        w1h, w1l = _bf16_split(w1t, 2)
        common["w1h"], common["w1l"] = w1h, w1l
    if W2_TERMS == 0:
        common["w2t"] = w2t
    else:
        for i, w in enumerate(_bf16_split(w2t, W2_TERMS)):
            common[f"w2s{i}"] = w

    in_maps = []
    for c in range(N_CORES):
        bs = slice(c * B_LOC, (c + 1) * B_LOC)
        xc = np.ascontiguousarray(x[bs, :T].transpose(2, 1, 0))   # (F,T,B)
        m0 = np.ascontiguousarray(hs[bs, 0].transpose(1, 2, 0))   # (2,H,B)
        m = dict(common)
        m["mem0"] = m0
        if W1_TERMS == 0:
            m["x"] = xc
        else:
            xh = xc.astype(ml_dtypes.bfloat16)
            xl = (xc - xh.astype(np.float32)).astype(ml_dtypes.bfloat16)
            m["xh"], m["xl"] = xh, xl
        in_maps.append(m)
    return in_maps


def assemble_output(results, B, T):
    out = np.empty((B, T, A), np.float32)
    for c in range(len(results)):
        bs = slice(c * B_LOC, (c + 1) * B_LOC)
        out[bs] = results[c]["out"].transpose(2, 1, 0)            # (B,T,A)
    return out


_NC_CACHE = {}


def kernel(**inputs) -> np.ndarray:
    x = np.asarray(inputs["state_batch"], np.float32)
    B, T, _ = x.shape
    if T not in _NC_CACHE:
        _NC_CACHE[T] = build_program(T)
    nc = _NC_CACHE[T]
    in_maps = make_in_maps(**inputs, T=T)
    res = run_bass_kernel_spmd(nc, in_maps, core_ids=list(range(N_CORES)),
                               trace=False)
    return assemble_output(res.results, B, T)


# revision 15
# speedup vs baseline: 1.0268x; 1.0268x over previous
"""Trainium2 Bass kernel for a 2-layer LIF spiking network (DSQN forward).

Math (per batch b, feature h, timestep t; THR=1, snntorch reset='subtract'):
    cur1 = W1 @ x_t + b1                      (precomputable, no recurrence)
    mem1 <- beta1*mem1 + cur1 - H(mem1 - 1)   (reset uses PREVIOUS mem)
    spk1 = H(mem1 - 1)
    cur2 = W2 @ spk1 + b2
    mem2 <- beta2*mem2 + cur2 - H(mem2 - 1)
    spk2 = H(mem2 - 1)
    out_t = W3 @ spk2 + b3

Design (v2 - raw bass, no Tile):
  - Pure data parallel: batch 512 -> 64 per core across 8 cores.
  - Feature-major on chip: partitions = H (128), free = (t, b) columns.
  - ONE fused custom DVE op per timestep advances BOTH layers (layer 2
    lagged by LAG chunks): tile [128, 2x64]: cols 0:64 = mem1_t,
    64:128 = mem2_{t-8*LAG}.  beta selected per column-half via Idx>=64.
    1024 ops x ~320ns = the DVE wall (~330us), semaphore-free chain
    (raw bass; same-engine FIFO gives the ordering, unlike Tile which
    pays a ~300ns/op semaphore round-trip).
  - Currents live in PSUM, interleaved per step [cur1_t | cur2_t];
    biases are injected by K=1 matmuls so the DVE reads PSUM directly
    (measured free) and no ScalarE PSUM->SBUF copy is needed.
  - Chunk buffer is split [mem1 steps | mem2 steps] so the bulk SIGN
    extraction and the W2/W3 matmul rhs stay contiguous (strided rhs
    halves TensorE throughput; strided dst + strided DVE io are free).
  - Matmuls in bf16 hi/lo splits (exact +-1 trick for W2/W3):
    W1: w1h@xh + w1h@xl + w1l@xh, W2: split-2, W3 single bf16.
  - Output bias c3 is added by the ScalarE PSUM->SBUF out-copy (free).
"""

from contextlib import ExitStack

import numpy as np
import ml_dtypes

import concourse.bacc as bacc
import concourse.mybir as mybir
from concourse.bass_utils import run_bass_kernel_spmd
from concourse.dve_spec import Spec, Src0, Src1, C0, C1, C2, One, Idx, lower
from concourse.dve_uop import DveOpSpec
from concourse.dve_ops import DveOp, OPS, _CUSTOM_DVE_ROW_BASE, _SUB_OPCODE_FOR_NAME

F32 = mybir.dt.float32
BF16 = mybir.dt.bfloat16
AF = mybir.ActivationFunctionType

N_CORES = 8
H = 128
F = 128
A = 16
B_LOC = 64          # batch per core
TC = 8              # timesteps per chunk
LAG = 3             # layer-2 lag in chunks
COLS1 = TC * B_LOC  # 512: one layer's chunk columns
NPC = 3             # pscomb PSUM slots (2 banks each) + 2 x p3 = 8 banks
NXS = 4             # x chunk slots
NCB = 3             # chunk buffer slots
NSG = 5             # sgn slots (> LAG+1)
NOB = 4             # out buffer slots

# Precision config: number of bf16 split terms (0 = fp32 matmul)
W1_TERMS = 0
W2_TERMS = 2
NBIAS = 6           # combined bias matmul K (3 bf16 split levels x 2 layers)


def _register_fused_op() -> DveOp:
    """Fused 2-layer LIF step on a [128, 2*64] tile.

    out = Src0 * beta + Src1 - (Src0 > 1),
    beta[k] = s0[p] for k<64 (layer1), s1[p] for k>=64 (layer2).
    """
    name = "LIF_FUSED2_ANT"
    for o in OPS:
        if o.name == name:
            return o
    beta = C0 + (Idx >= C2) * (C1 - C0)
    body = Src0 * beta + Src1 - (Src0 > One)

    def ref(in0, in1, s0, s1, imm2):
        i0 = in0.reshape(in0.shape[0], -1)
        i1 = in1.reshape(in1.shape[0], -1)
        idx = np.arange(i0.shape[1])[None, :]
        b = np.where(idx >= imm2, s1, s0)
        return (i0 * b + i1 - (i0 > 1.0)).astype(np.float32).reshape(in0.shape)

    spec = Spec(body=body, reference=ref)
    shas = {
        ver: DveOpSpec(name=name, uops=lower(spec, ver=ver), rd1_en=True).sha(ver)
        for ver in ("v3", "v4")
    }
    op = DveOp(name, spec, subdim=False, uops_sha=shas)
    OPS.append(op)
    _SUB_OPCODE_FOR_NAME[name] = _CUSTOM_DVE_ROW_BASE + len(OPS) - 1
    return op


def build_program(T: int = 1024):
    assert T % TC == 0
    NCH = T // TC              # main chunks (mem1 timeline)
    VCH = NCH + LAG            # vector chunks incl. layer-2 drain
    OUT_SHIFT = 10             # sync issues out-DMA[c - OUT_SHIFT]
    fused = _register_fused_op()

    nc = bacc.Bacc("TRN2", target_bir_lowering=False, debug=False,
                   num_devices=N_CORES, disable_frame_to_traceback=True)

    # ---- DRAM ----
    if W1_TERMS == 0:
        x_d = nc.dram_tensor("x", (F, T, B_LOC), F32, kind="ExternalInput")
        w1_d = nc.dram_tensor("w1t", (F, H), F32, kind="ExternalInput")
        XD = 1
    else:
        xh_d = nc.dram_tensor("xh", (F, T, B_LOC), BF16, kind="ExternalInput")
        xl_d = nc.dram_tensor("xl", (F, T, B_LOC), BF16, kind="ExternalInput")
        w1h_d = nc.dram_tensor("w1h", (F, H), BF16, kind="ExternalInput")
        w1l_d = nc.dram_tensor("w1l", (F, H), BF16, kind="ExternalInput")
        XD = 2
    if W2_TERMS == 0:
        w2_d = nc.dram_tensor("w2t", (H, H), F32, kind="ExternalInput")
    else:
        w2s_d = [nc.dram_tensor(f"w2s{i}", (H, H), BF16, kind="ExternalInput")
                 for i in range(W2_TERMS)]
    w3_d = nc.dram_tensor("w3t", (H, A), BF16, kind="ExternalInput")
    m0_d = nc.dram_tensor("mem0", (2, H, B_LOC), F32, kind="ExternalInput")
    brows_d = nc.dram_tensor("brows", (NBIAS, H), BF16, kind="ExternalInput")
    bmask_d = nc.dram_tensor("bmask", (NBIAS, COLS1), BF16, kind="ExternalInput")
    beta1_d = nc.dram_tensor("beta1", (H, 1), F32, kind="ExternalInput")
    beta2_d = nc.dram_tensor("beta2", (H, 1), F32, kind="ExternalInput")
    c3_d = nc.dram_tensor("c3", (A, 1), F32, kind="ExternalInput")
    neg1_d = nc.dram_tensor("neg1", (H, 1), F32, kind="ExternalInput")
    out_d = nc.dram_tensor("out", (A, T, B_LOC), F32, kind="ExternalOutput")

    es = ExitStack()
    with es:
        block = es.enter_context(nc.Block())
        sems = [es.enter_context(nc.semaphore(n)) for n in
                ("s_pre", "s_x", "s_ps", "s_vec", "s_sc", "s_w3", "s_oc",
                 "s_od")]
        s_pre, s_x, s_ps, s_vec, s_sc, s_w3, s_oc, s_od = sems

        def sbuf(name, shape, dt):
            return es.enter_context(nc.sbuf_tensor(name, shape, dt))

        w3_s = sbuf("w3s", [H, A], BF16)
        brows_s = sbuf("brows_s", [NBIAS, H], BF16)
        bmask_s = sbuf("bmask_s", [NBIAS, COLS1], BF16)
        beta1_s = sbuf("beta1_s", [H, 1], F32)
        beta2_s = sbuf("beta2_s", [H, 1], F32)
        c3_s = sbuf("c3_s", [A, 1], F32)
        neg1_s = sbuf("neg1_s", [H, 1], F32)
        minit = sbuf("minit", [128, 2 * COLS1], F32)
        cb = sbuf("cb", [128, NCB * 2 * COLS1], F32)
        sgf = sbuf("sgf", [128, NSG * COLS1], BF16)
        sgb = sbuf("sgb", [128, NSG * COLS1], BF16)
        ob = sbuf("ob", [A, NOB * COLS1], F32)
        pc0 = es.enter_context(nc.psum_tensor("pc0", [128, 2 * COLS1], F32))
        pc1 = es.enter_context(nc.psum_tensor("pc1", [128, 2 * COLS1], F32))
        pc2 = es.enter_context(nc.psum_tensor("pc2", [128, 2 * COLS1], F32))
        p30 = es.enter_context(nc.psum_tensor("p30", [128, COLS1], F32))
        p31 = es.enter_context(nc.psum_tensor("p31", [128, COLS1], F32))
        pre_dmas = []

        def pre(dst, src):
            pre_dmas.append((dst, src))

        if W1_TERMS == 0:
            w1_s = sbuf("w1s", [F, H], F32)
            x_s = sbuf("xs", [F, NXS * COLS1], F32)
            pre(w1_s[:], w1_d.ap())
        else:
            w1h_s = sbuf("w1hs", [F, H], BF16)
            w1l_s = sbuf("w1ls", [F, H], BF16)
            xh_s = sbuf("xhs", [F, NXS * COLS1], BF16)
            xl_s = sbuf("xls", [F, NXS * COLS1], BF16)
            pre(w1h_s[:], w1h_d.ap())
            pre(w1l_s[:], w1l_d.ap())
        if W2_TERMS == 0:
            w2_s = sbuf("w2fs", [H, H], F32)
            pre(w2_s[:], w2_d.ap())
            w2list = [w2_s]
        else:
            w2list = []
            for i in range(W2_TERMS):
                t = sbuf(f"w2s{i}s", [H, H], BF16)
                w2list.append(t)
                pre(t[:], w2s_d[i].ap())
        pre(w3_s[:], w3_d.ap())
        pre(brows_s[:], brows_d.ap())
        pre(bmask_s[:], bmask_d.ap())
        pre(beta1_s[:], beta1_d.ap())
        pre(beta2_s[:], beta2_d.ap())
        pre(c3_s[:], c3_d.ap())
        pre(neg1_s[:], neg1_d.ap())
        # initial membranes into the strided carry layout: step-7 blocks
        pre(minit[:, 7 * B_LOC:8 * B_LOC], m0_d.ap()[0])
        pre(minit[:, COLS1 + 7 * B_LOC:COLS1 + 8 * B_LOC], m0_d.ap()[1])
        PRE = 16 * len(pre_dmas)

        pcs = [pc0, pc1, pc2]
        p3s = [p30, p31]

        # cb as [p, slot, half, step, col]
        cb5 = cb.ap().rearrange("p (s h u c) -> p s h u c",
                                s=NCB, h=2, u=TC, c=B_LOC)
        mi4 = minit.ap().rearrange("p (h u c) -> p h u c", h=2, u=TC, c=B_LOC)

        def carry_ap(slot, t):
            return cb5[:, slot, :, t, :]

        def pc_dst(pcten, half, hb):
            # [p, step, 64] strided dst for matmuls (even/odd 64-blocks),
            # restricted to PSUM bank hb (a matmul dst cannot span banks)
            return pcten.ap().rearrange("p (hb t h c) -> p hb t h c",
                                        hb=2, t=TC // 2, h=2,
                                        c=B_LOC)[:, hb, :, half, :]

        def x_chunk(xd, c):
            return xd.ap()[:, c * TC:(c + 1) * TC, :].rearrange(
                "p a b -> p (a b)")

        def sg1_ap(c):
            o = (c % NSG) * COLS1
            return sgf.ap()[:, o:o + COLS1]

        def sg2_ap(c):
            o = (c % NSG) * COLS1
            return sgb.ap()[:, o:o + COLS1]

        def out_dma(sync, o):
            sync.wait_ge(s_oc, o + 1)
            sync.dma_start(
                out_d.ap()[:, o * TC:(o + 1) * TC, :].rearrange(
                    "p a b -> p (a b)"),
                ob[:, (o % NOB) * COLS1:(o % NOB + 1) * COLS1],
            ).then_inc(s_od, 16)

        # ---------------- SYNC (DMA) ----------------
        @block.sync
        def _(sync):
            for dst, src in pre_dmas:
                sync.dma_start(dst, src).then_inc(s_pre, 16)
            for c in range(NCH):
                if c >= NXS:
                    sync.wait_ge(s_ps, c - NXS + 1)
                xs = slice((c % NXS) * COLS1, (c % NXS + 1) * COLS1)
                if W1_TERMS == 0:
                    sync.dma_start(x_s[:, xs], x_chunk(x_d, c)).then_inc(s_x, 16)
                else:
                    sync.dma_start(xh_s[:, xs], x_chunk(xh_d, c)).then_inc(s_x, 16)
                    sync.dma_start(xl_s[:, xs], x_chunk(xl_d, c)).then_inc(s_x, 16)
                if c - OUT_SHIFT >= 0:
                    out_dma(sync, c - OUT_SHIFT)
            for o in range(max(0, NCH - OUT_SHIFT), NCH):
                out_dma(sync, o)

        # ---------------- TENSOR ----------------
        @block.tensor
        def _(tensor):
            tensor.wait_ge(s_pre, PRE)
            n_w3 = 0

            def w3_mm(k):
                tensor.wait_ge(s_sc, k + 1)          # sgn2[k] ready
                if k - 4 > 0:
                    tensor.wait_ge(s_oc, k - 4)      # p3 slot free
                nc.tensor.matmul(p3s[k % 2].ap()[0:A, :], w3_s[:], sg2_ap(k),
                                 start=True, stop=True).then_inc(s_w3, 1)

            for c in range(VCH):
                pc = pcs[c % NPC]
                HB = COLS1 // 2  # 256 columns per PSUM bank per half
                if c < NCH:
                    tensor.wait_ge(s_x, 16 * XD * (c + 1))
                if c >= NPC:
                    tensor.wait_ge(s_vec, c - NPC + 1)  # pscomb slot free
                if c >= LAG:
                    tensor.wait_ge(s_sc, c - LAG + 1)   # sgn1[c-LAG] ready
                ins = None
                for hb in range(2):
                    bank = pc.ap()[:, hb * COLS1:(hb + 1) * COLS1]
                    # combined bias fill for both layers (starts the group)
                    ins = nc.tensor.matmul(bank, brows_s[:, :], bmask_s[:, :],
                                           start=True,
                                           stop=(c >= NCH and c < LAG))
                    x0 = (c % NXS) * COLS1 + hb * HB
                    hs_ = slice(hb * HB, (hb + 1) * HB)
                    if c < NCH:
                        ins = nc.tensor.matmul(pc_dst(pc, 0, hb), w1_s[:],
                                               x_s[:, x0:x0 + HB],
                                               start=False, stop=(c < LAG))
                if c >= LAG:
                    sg1c = sg1_ap(c - LAG)
                    for i, w2t in enumerate(w2list):
                        for hb in range(2):
                            hs_ = slice(hb * HB, (hb + 1) * HB)
                            ins = nc.tensor.matmul(pc_dst(pc, 1, hb), w2t[:],
                                                   sg1c[:, hs_], start=False,
                                                   stop=(i == len(w2list) - 1))
                ins.then_inc(s_ps, 1)
                # ---- W3 for k = c-3 (lagged so its sgn2 wait is already
                # satisfied by the bank-1 wait and never blocks the pscomb
                # stream the vector chain depends on) ----
                k = c - 3
                if k >= LAG:
                    w3_mm(k)
                    n_w3 += 1
            for k in (VCH - 3, VCH - 2, VCH - 1):
                w3_mm(k)
                n_w3 += 1
            assert n_w3 == NCH

        # ---------------- VECTOR (the serial LIF chain) ----------------
        @block.vector
        def _(vector):
            vector.wait_ge(s_pre, PRE)
            for c in range(VCH):
                vector.wait_ge(s_ps, c + 1)
                if c >= NCB:
                    vector.wait_ge(s_sc, c - NCB + 2)  # cb slot free
                pc = pcs[c % NPC]
                slot = c % NCB
                for t in range(TC):
                    if t == 0:
                        src = (mi4[:, :, TC - 1, :] if c == 0
                               else carry_ap((c - 1) % NCB, TC - 1))
                    else:
                        src = carry_ap(slot, t - 1)
                    ins = nc.vector._custom_dve(
                        fused,
                        out=carry_ap(slot, t),
                        in0=src,
                        in1=pc.ap()[:, t * 2 * B_LOC:(t + 1) * 2 * B_LOC],
                        s0=beta1_s[:, 0:1],
                        s1=beta2_s[:, 0:1],
                        imm2=float(B_LOC),
                    )
                if c == LAG - 1:
                    # layer-2 carry re-init: garbage evolved during the lag
                    # warmup; restore the true initial mem2 before chunk LAG
                    ins = nc.vector.tensor_copy(
                        cb5[:, slot, 1, TC - 1, :], mi4[:, 1, TC - 1, :])
                ins.then_inc(s_vec, 1)

        # ---------------- SCALAR ----------------
        @block.scalar
        def _(scalar):
            scalar.wait_ge(s_pre, PRE)
            n_oc = 0

            def outcopy(k):
                o = k - LAG
                scalar.wait_ge(s_w3, k - LAG + 1)     # p3[k] written
                if o - NOB >= 0:
                    scalar.wait_ge(s_od, 16 * (o - NOB + 1))  # ob slot free
                nc.scalar.activation(
                    ob[:, (o % NOB) * COLS1:(o % NOB + 1) * COLS1],
                    p3s[k % 2].ap()[0:A, :],
                    AF.Identity, bias=c3_s[:, 0:1]).then_inc(s_oc, 1)

            for c in range(VCH):
                scalar.wait_ge(s_vec, c + 1)
                if c >= NSG:
                    scalar.wait_ge(s_ps, c - NSG + LAG + 2)  # sg slot free
                slot = c % NCB
                ins = None
                if c < NCH:
                    ins = nc.scalar.activation(
                        sg1_ap(c),
                        cb5[:, slot, 0, :, :].rearrange("p u c -> p (u c)"),
                        AF.Sign, bias=neg1_s[:, 0:1])
                if c >= LAG:
                    ins = nc.scalar.activation(
                        sg2_ap(c),
                        cb5[:, slot, 1, :, :].rearrange("p u c -> p (u c)"),
                        AF.Sign, bias=neg1_s[:, 0:1])
                assert ins is not None
                ins.then_inc(s_sc, 1)
                # out-copy for k = c-3 (mirrors the tensor W3 lag)
                k = c - 3
                if k >= LAG:
                    outcopy(k)
                    n_oc += 1
            for k in (VCH - 3, VCH - 2, VCH - 1):
                outcopy(k)
                n_oc += 1
            assert n_oc == NCH

    nc.compile()
    # Path-independent debug info so the neuron compile cache hits across
    # working directories.
    fixed_dbg = mybir.OpDebugInfo(filename="kernel.py", lineno=0,
                                  kernel_name="build_program:")
    _dbg_cache = {}

    def _sanitize(dbg):
        if dbg is None:
            return None
        key = (dbg.op_name, dbg.lineno, dbg.bass_funcname, dbg.kernel_name,
               dbg.ant_layer, dbg.ant_annotation)
        if key not in _dbg_cache:
            _dbg_cache[key] = mybir.OpDebugInfo(
                op_name=dbg.op_name, filename="kernel.py", lineno=dbg.lineno,
                bass_funcname=dbg.bass_funcname, kernel_name=dbg.kernel_name,
                ant_layer=dbg.ant_layer, ant_annotation=dbg.ant_annotation)
        return _dbg_cache[key]

    for fn in nc.m.functions:
        for alloc in fn.allocations:
            for ml in (getattr(alloc, "memorylocations", None) or []):
                if getattr(ml, "ant_debug", None) is not None:
                    ml.ant_debug = fixed_dbg
        for blk in fn.blocks:
            for inst in blk.instructions:
                inst.debug = _sanitize(inst.debug)
    return nc


def _bf16_split(a, terms):
    """Split fp32 array into `terms` bf16 arrays summing to ~a."""
    outs = []
    rem = np.asarray(a, np.float32)
    for _ in range(terms):
        h = rem.astype(ml_dtypes.bfloat16)
        outs.append(h)
        rem = (rem - h.astype(np.float32)).astype(np.float32)
    return outs


def make_in_maps(state_batch, hidden_states, W1, b1, beta1, W2, b2, beta2,
                 W3, b3, T=None):
    x = np.asarray(state_batch, np.float32)
    hs = np.asarray(hidden_states, np.float32)
    if T is None:
        T = x.shape[1]
    W1 = np.asarray(W1, np.float32)
    W2 = np.asarray(W2, np.float32)
    W3 = np.asarray(W3, np.float32)

    w1t = np.ascontiguousarray(W1.T)                       # (F,H)
    w2t = np.ascontiguousarray((0.5 * W2).T)               # (H,H)
    w3t = np.ascontiguousarray((0.5 * W3).T).astype(ml_dtypes.bfloat16)
    be1 = np.clip(np.asarray(beta1, np.float32), 0.0, 1.0).reshape(H, 1)
    be2 = np.clip(np.asarray(beta2, np.float32), 0.0, 1.0).reshape(H, 1)
    b1row = np.asarray(b1, np.float32).reshape(H)
    b2row = (np.asarray(b2, np.float64)
             + 0.5 * np.asarray(W2, np.float64).sum(1)).astype(
                 np.float32).reshape(H)
    brows = np.stack(_bf16_split(b1row, 3) + _bf16_split(b2row, 3))  # (6,H)
    blk = (np.arange(COLS1) // B_LOC) % 2
    bmask = np.zeros((NBIAS, COLS1), np.float32)
    bmask[0:3] = (blk == 0).astype(np.float32)[None, :]
    bmask[3:6] = (blk == 1).astype(np.float32)[None, :]
    bmask = bmask.astype(ml_dtypes.bfloat16)
    c3 = (np.asarray(b3, np.float64)
          + 0.5 * np.asarray(W3, np.float64).sum(1)).astype(
              np.float32).reshape(A, 1)
    neg1 = np.full((H, 1), -1.0, np.float32)

    common = {
        "w3t": w3t, "brows": brows, "bmask": bmask,
        "beta1": be1, "beta2": be2, "c3": c3, "neg1": neg1,
    }
    if W1_TERMS == 0:
        common["w1t"] = w1t
    else:
        w1h, w1l = _bf16_split(w1t, 2)
        common["w1h"], common["w1l"] = w1h, w1l
    if W2_TERMS == 0:
        common["w2t"] = w2t
    else:
        for i, w in enumerate(_bf16_split(w2t, W2_TERMS)):
            common[f"w2s{i}"] = w

    in_maps = []
    for c in range(N_CORES):
        bs = slice(c * B_LOC, (c + 1) * B_LOC)
        xc = np.ascontiguousarray(x[bs, :T].transpose(2, 1, 0))   # (F,T,B)
        m0 = np.ascontiguousarray(hs[bs, 0].transpose(1, 2, 0))   # (2,H,B)
        m = dict(common)
        m["mem0"] = m0
        if W1_TERMS == 0:
            m["x"] = xc
        else:
            xh = xc.astype(ml_dtypes.bfloat16)
            xl = (xc - xh.astype(np.float32)).astype(ml_dtypes.bfloat16)
            m["xh"], m["xl"] = xh, xl
        in_maps.append(m)
    return in_maps


def assemble_output(results, B, T):
    out = np.empty((B, T, A), np.float32)
    for c in range(len(results)):
        bs = slice(c * B_LOC, (c + 1) * B_LOC)
        out[bs] = results[c]["out"].transpose(2, 1, 0)            # (B,T,A)
    return out


_NC_CACHE = {}


def kernel(**inputs) -> np.ndarray:
    x = np.asarray(inputs["state_batch"], np.float32)
    B, T, _ = x.shape
    if T not in _NC_CACHE:
        _NC_CACHE[T] = build_program(T)
    nc = _NC_CACHE[T]
    in_maps = make_in_maps(**inputs, T=T)
    res = run_bass_kernel_spmd(nc, in_maps, core_ids=list(range(N_CORES)),
                               trace=False)
    return assemble_output(res.results, B, T)
